# revision 33
# baseline (speedup 1.0000x reference)
"""Trainium2 Bass kernel for segmented linear (performer-style) attention.

Problem: nn_Attention_43550968382196 (sparse_attention).
  N=32768 tokens in 64 contiguous equal segments of 512, d_qk=128, d_v=256,
  m=256 random features.  Per segment:
     phi_q = (exp(Uq - hq - rowmax(Uq)) + eps) / sqrt(m)
     phi_k = (exp(Uk - hk - segmax(Uk)) + eps) / sqrt(m)
     out   = (phi_q @ (phi_k^T V)) / (phi_q . sum(phi_k) + 1e-8)

Device math (v8; validated ~5e-3 rel err vs the jax reference in numpy):
  * All matmuls bf16 operands, fp32 PSUM accumulation (fp8 was tested and
    fails the 2e-2 gate: e4m3's 6% per-element error survives averaging).
  * Q side: Qp = exp(Uq - hq - mx) via one Act pass per chunk (bias AP);
    the +eps rides the PSUM->SBUF copy after the PE transpose (Copy with
    float bias / tensor_scalar_add), so no eps rank-1 on the Q side.
  * K side: e^{-hk} is folded into V' rows ON THE HOST, and V' carries a
    257th column equal to e^{-hk}, so phi_k-dev = exp(Uk) needs NO bias
    (one exp per psum tile) and Ksum drops out of the KV matmul as column
    256 -- no separate Ksum or den matmul chains.  The segment max enters
    only through the eps correction: segmax' = max(exp(Uk)) = e^{segmax}
    via a gpsimd all-dims reduce, used as the rank-1 lhsT scale:
    KV += segmax' * (eps * [colsum_raw(V) | 512]).  The per-segment scale
    e^{segmax} cancels in the num/den ratio.
  * num[:, 0:256] and den = num[:, 256] leave as one bf16 [512, 257] DMA
    per segment; the division (+ the 1e-8*m*segmax' norm epsilon) happens
    on the host.
  * ~5us of dummy matmuls at t=0 ramp the PE p-state to 2.4GHz while the
    input DMAs fill SBUF.

Sharding: 64 segments split 8-per-core across 8 NeuronCores (data parallel,
no collectives); each core runs this program on its 4096-token shard.
"""

import math
import os
import sys

for _p in ("/opt/trn_rl_repo",):
    if _p not in sys.path and os.path.isdir(_p):
        sys.path.insert(0, _p)

import numpy as np
import ml_dtypes

import concourse.bass as bass
import concourse.bacc as bacc
import concourse.tile as tile
from concourse import mybir
from concourse.bass_utils import run_bass_kernel_spmd

F32 = mybir.dt.float32
BF16 = mybir.dt.bfloat16
AF = mybir.ActivationFunctionType
ALU = mybir.AluOpType
AX = mybir.AxisListType

N_CORES = 8
N = 32768
D = 128          # qk dim
M = 256          # features
DV = 256         # v dim
DV1 = 257        # device V' columns: [V | 1] (all rows scaled by e^{-hk})
P = 128          # partitions / tokens per chunk
NSEG = 8         # segments per core
CH = 4           # chunks per segment
MC = 2           # m chunks (256 / 128)
TOK = NSEG * 512

EPS_PHI = 1e-4
EPS_NORM = 1e-8


def build_nc():
    nc = bacc.Bacc("TRN2", target_bir_lowering=False, debug=False)

    QTd = nc.declare_dram_parameter("QT", [D, TOK], BF16, isOutput=False)
    KTd = nc.declare_dram_parameter("KT", [D, TOK], BF16, isOutput=False)
    Vd = nc.declare_dram_parameter("VP", [TOK, DV1], BF16, isOutput=False)
    Wd = nc.declare_dram_parameter("omega", [D, M], BF16, isOutput=False)
    HQd = nc.declare_dram_parameter("HQM", [P, NSEG * CH], F32, isOutput=False)
    CVd = nc.declare_dram_parameter("CVS", [1, NSEG * DV1], BF16,
                                    isOutput=False)
    Id = nc.declare_dram_parameter("identb", [P, P], BF16, isOutput=False)
    Od = nc.declare_dram_parameter("num", [TOK, DV1], BF16, isOutput=True)
    Sd = nc.declare_dram_parameter("smax", [1, NSEG], F32, isOutput=True)

    Vv = Vd[:, :].rearrange("(s c p) d -> s p c d", s=NSEG, c=CH, p=P)
    Ov = Od[:, :].rearrange("(s c p) d -> s p c d", s=NSEG, c=CH, p=P)

    with tile.TileContext(nc) as tc:
        with (
            tc.tile_pool(name="const", bufs=1) as const,
            tc.tile_pool(name="sb", bufs=2) as sb,
            tc.tile_pool(name="sm", bufs=3) as sm,
            tc.tile_pool(name="ps", bufs=1, space="PSUM") as ps,
        ):
            # PE warm-up scratch (no input deps)
            scr1 = const.tile([P, 1], BF16, name="scr1")
            nc.vector.memset(scr1[:, :], 1.0)
            scr2 = const.tile([P, 512], BF16, name="scr2")
            nc.vector.memset(scr2[:, :], 1.0)

            # omega + segment-0 inputs first, then consts, then bulk rest
            qT_all = const.tile([D, TOK], BF16, name="qT_all")
            kT_all = const.tile([D, TOK], BF16, name="kT_all")
            vp_all = const.tile([P, NSEG, CH, DV1], BF16, name="vp_all")
            omega_t = const.tile([D, M], BF16, name="omega_t")
            nc.sync.dma_start(omega_t[:, :], Wd[:, :])
            nc.sync.dma_start(qT_all[:, 0:512], QTd[:, 0:512])
            nc.sync.dma_start(kT_all[:, 0:512], KTd[:, 0:512])
            ident_t = const.tile([P, P], BF16, name="ident_t")
            nc.sync.dma_start(ident_t[:, :], Id[:, :])
            nc.sync.dma_start(vp_all[:, 0], Vv[0])
            hqm_t = const.tile([P, NSEG, CH], F32, name="hqm_t")
            nc.sync.dma_start(
                hqm_t[:, :, :],
                HQd[:, :].rearrange("p (s c) -> p s c", s=NSEG))
            cvs_t = const.tile([1, NSEG, DV1], BF16, name="cvs_t")
            nc.sync.dma_start(
                cvs_t[:, :, :],
                CVd[:, :].rearrange("p (s d) -> p s d", s=NSEG))
            smaxAll = const.tile([1, NSEG], F32, name="smaxAll")

            # remaining per-segment loads (keeps early segments' data
            # close); segment 1 issues from the idle Act HWDGE queue so
            # its transfers run in parallel with SP's during the fill
            for s in range(1, NSEG):
                sl = bass.ts(s, 512)
                eng = nc.scalar if s == 1 else nc.sync
                eng.dma_start(qT_all[:, sl], QTd[:, sl])
                eng.dma_start(kT_all[:, sl], KTd[:, sl])
                eng.dma_start(vp_all[:, s], Vv[s])

            # warm-up matmuls during the DMA fill (output unread)
            warm = ps.tile([P, DV1], F32, name="warm", tag="NN", bufs=2)
            for i in range(9):
                nc.tensor.matmul(warm[0:1, :], scr1[:, 0:1],
                                 scr2[:, 0:DV1], skip_group_check=True)

            # per-segment state carried between pipeline stages
            stK = [None] * NSEG
            st = [None] * NSEG

            def stageK(s):
                # ---- K side, run 2 segments ahead: hides the smax
                # chain (expK -> gpsimd 2us -> smrow) completely -------
                uk0 = ps.tile([P, 2, M], F32, name=f"uk0_{s}", tag="U", bufs=4)
                uk1 = ps.tile([P, 2, M], F32, name=f"uk1_{s}", tag="U", bufs=4)
                for c in range(CH):
                    u = (uk0, uk1)[c // 2]
                    nc.tensor.matmul(u[:, c % 2, :],
                                     kT_all[:, bass.ts(s * CH + c, P)],
                                     omega_t[:, :])
                # K: exp with no bias (one op per psU tile)
                kp = sb.tile([P, CH, M], BF16, name=f"kp{s}", tag="kp", bufs=4)
                nc.scalar.activation(kp[:, 0:2, :], uk0[:, :, :], AF.Exp)
                nc.scalar.activation(kp[:, 2:4, :], uk1[:, :, :], AF.Exp)
                # segmax' = max(exp(Uk)) via gpsimd all-reduce (SBUF in)
                smx = sm.tile([1, 1], F32, name=f"smx{s}", tag="smx")
                nc.gpsimd.tensor_reduce(smx[:, :], kp[:, :, :],
                                        axis=AX.XYZWC, op=ALU.max)
                smrow = sm.tile([1, P], BF16, name=f"smrow{s}", tag="smrow")
                nc.vector.tensor_copy(smrow[:, :],
                                      smx[:, :].broadcast_to([1, P]))
                nc.gpsimd.tensor_copy(smaxAll[0:1, s:s + 1], smx[:, :])
                stK[s] = (kp, smrow)

            def stage1_mm(s):
                # ---- Q side: U matmuls, rowmax -> bias ---------------
                uq0 = ps.tile([P, 2, M], F32, name=f"uq0_{s}", tag="U", bufs=4)
                uq1 = ps.tile([P, 2, M], F32, name=f"uq1_{s}", tag="U", bufs=4)
                for c in range(CH):
                    u = (uq0, uq1)[c // 2]
                    nc.tensor.matmul(u[:, c % 2, :],
                                     qT_all[:, bass.ts(s * CH + c, P)],
                                     omega_t[:, :])
                mx4 = sm.tile([P, CH], F32, name=f"mx4_{s}", tag="mx4")
                nc.vector.tensor_reduce(mx4[:, 0:2], uq0[:, :, :],
                                        axis=AX.X, op=ALU.max)
                nc.vector.tensor_reduce(mx4[:, 2:4], uq1[:, :, :],
                                        axis=AX.X, op=ALU.max)
                biasq = sm.tile([P, CH], F32, name=f"biasq_{s}", tag="biasq")
                nc.gpsimd.tensor_tensor(biasq[:, :], hqm_t[:, s], mx4[:, :],
                                        op=ALU.subtract)
                st[s] = (uq0, uq1, biasq)

            def stage1_exp(s):
                uq0, uq1, biasq = st[s]
                qp = sb.tile([P, CH, M], BF16, name=f"qp{s}", tag="qp", bufs=3)
                for c in range(CH):
                    nc.scalar.activation(qp[:, c, :],
                                         (uq0, uq1)[c // 2][:, c % 2, :],
                                         AF.Exp, bias=biasq[:, c:c + 1])
                st[s] = qp

            qpTs = [None] * NSEG
            kvbs = [None] * NSEG

            def stage2a_T(s):
                qp = st[s]
                # ---- QpT = T(qp) + eps  (PE transpose, copy adds eps) -
                psT = ps.tile([P, MC, 512], BF16, name=f"psT_{s}", tag="T",
                              bufs=1)
                for c in range(CH):
                    nc.tensor.transpose(psT[:, 0, bass.ts(c, P)],
                                        qp[:, c, 0:P], ident_t[:, :])
                    nc.tensor.transpose(psT[:, 1, bass.ts(c, P)],
                                        qp[:, c, P:M], ident_t[:, :])
                qpT = sb.tile([P, MC, 512], BF16, name=f"qpT{s}", tag="qpT",
                              bufs=2)
                nc.scalar.activation(qpT[:, 0, :], psT[:, 0, :], AF.Copy,
                                     bias=EPS_PHI)
                nc.vector.tensor_scalar_add(qpT[:, 1, :], psT[:, 1, :],
                                            EPS_PHI)
                qpTs[s] = qpT
                kvbs[s] = sb.tile([P, MC, DV1], BF16, name=f"kvb{s}",
                                  tag="kvb", bufs=2)

            def stage2a_KV(s, mc):
                kp, smrow = stK[s]
                # ---- KV = Kp^T [V'|e^{-hk}] (+ rank-1 eps, first) -----
                psKV = ps.tile([P, DV1], F32, name=f"psKV{s}_{mc}",
                               tag="W", bufs=1)
                nc.tensor.matmul(psKV[:, :], smrow[0:1, :],
                                 cvs_t[0:1, s, :], start=True, stop=False)
                for c in range(CH):
                    nc.tensor.matmul(psKV[:, :],
                                     kp[:, c, bass.ts(mc, P)],
                                     vp_all[:, s, c, :],
                                     start=False, stop=(c == CH - 1))
                if mc == 0:
                    nc.vector.tensor_copy(kvbs[s][:, 0, :], psKV[:, :])
                else:
                    nc.scalar.activation(kvbs[s][:, 1, :], psKV[:, :],
                                         AF.Copy)

            def stage2b(s):
                qpT, kvb = qpTs[s], kvbs[s]
                # ---- num matmuls ([t, V'|den] per chunk) + store ------
                for half in range(2):
                    numb = sb.tile([P, 2, DV1], BF16,
                                   name=f"numb{s}_{half}", tag="numb",
                                   bufs=3)
                    for i in range(2):
                        c = half * 2 + i
                        psN = ps.tile([P, DV1], F32, name=f"psN{s}_{c}",
                                      tag="NN", bufs=2)
                        for mc in range(MC):
                            nc.tensor.matmul(psN[:, :],
                                             qpT[:, mc, bass.ts(c, P)],
                                             kvb[:, mc, :],
                                             start=(mc == 0), stop=(mc == 1))
                        if s == NSEG - 1 and i == 0:
                            # tail: parallelize the last segment's copies
                            nc.scalar.activation(numb[:, i, :], psN[:, :],
                                                 AF.Copy)
                        else:
                            nc.vector.tensor_copy(numb[:, i, :], psN[:, :])
                    nc.sync.dma_start(Ov[s, :, 2 * half:2 * half + 2, :],
                                      numb[:, :, :])

            # 2.5-deep software pipeline: iteration s emits Q-matmuls for
            # s+1, K side for s+2, transposes+KV for s, num for s-1 -- so
            # every PE instruction's deps are >= 1 iteration old.
            # Prefix: Q side first so biasq(0) isn't queued behind the
            # 2us gpsimd segmax reduce on Pool, and expQ(0) leads Act.
            stage1_mm(0)
            stage1_exp(0)
            stageK(0)
            stageK(1)
            for s in range(NSEG):
                if s + 1 < NSEG:
                    stage1_mm(s + 1)
                if s + 2 < NSEG:
                    stageK(s + 2)
                stage2a_T(s)
                # KV mc=1 early: its Act-side kvb copy gates the W-ring
                # recycle for the next segment, so don't queue it behind
                # the expQ block.
                stage2a_KV(s, 1)
                if s + 1 < NSEG:
                    stage1_exp(s + 1)
                stage2a_KV(s, 0)
                if s > 0:
                    stage2b(s - 1)
            stage2b(NSEG - 1)

            nc.sync.dma_start(Sd[:, :], smaxAll[:, :])

    nc.compile()
    return nc


_NC_CACHE = {}


def _get_nc():
    if "nc" not in _NC_CACHE:
        _NC_CACHE["nc"] = build_nc()
    return _NC_CACHE["nc"]


def make_in_maps(Q, K, V, omega):
    bf = ml_dtypes.bfloat16
    Q = np.ascontiguousarray(np.asarray(Q, dtype=np.float32))
    K = np.ascontiguousarray(np.asarray(K, dtype=np.float32))
    V = np.ascontiguousarray(np.asarray(V, dtype=np.float32))
    omega = np.asarray(omega, dtype=np.float32)

    QT = Q.T.astype(bf)
    KT = K.T.astype(bf)
    omega_s = (omega * np.float32(D ** -0.25)).astype(bf)
    hscale = np.float32(1.0 / (2.0 * math.sqrt(D)))
    hq = (Q * Q).sum(axis=1) * hscale            # [N]
    hk = (K * K).sum(axis=1) * hscale
    ehk = np.exp(-hk).astype(np.float32)          # [N]
    Vb = V.astype(bf).astype(np.float32)
    # V' = e^{-hk} * [V | 1]  (column 256 = e^{-hk} itself)
    VP = (ehk[:, None] * np.concatenate(
        [Vb, np.ones((N, 1), np.float32)], axis=1)).astype(bf)
    # eps * per-segment colsum of raw [V | 1] (bf16-rounded V)
    nseg_tot = N_CORES * NSEG
    cvs = np.concatenate(
        [EPS_PHI * Vb.reshape(nseg_tot, 512, DV).sum(axis=1),
         np.full((nseg_tot, 1), EPS_PHI * 512.0, np.float32)],
        axis=1).astype(bf)                        # [nseg, 257]
    ident = np.eye(P, dtype=np.float32).astype(bf)

    hqm = np.ascontiguousarray(
        (-hq).reshape(N_CORES, NSEG, CH, P).transpose(0, 3, 1, 2)
        .reshape(N_CORES, P, NSEG * CH)).astype(np.float32)

    in_maps = []
    for c in range(N_CORES):
        sl = slice(c * TOK, (c + 1) * TOK)
        in_maps.append({
            "QT": np.ascontiguousarray(QT[:, sl]),
            "KT": np.ascontiguousarray(KT[:, sl]),
            "VP": VP[sl],
            "omega": omega_s,
            "HQM": hqm[c],
            "CVS": np.ascontiguousarray(
                cvs[c * NSEG:(c + 1) * NSEG].reshape(1, NSEG * DV1)),
            "identb": ident,
        })
    return in_maps


def assemble(results):
    outs = []
    for c in range(N_CORES):
        r = results[c]
        num = np.asarray(r["num"], dtype=np.float32)          # [TOK, 257]
        smax = np.asarray(r["smax"], dtype=np.float32).reshape(NSEG)
        den = num[:, DV] + (M * EPS_NORM) * np.repeat(smax, 512)
        outs.append(num[:, 0:DV] / den[:, None])
    return np.concatenate(outs, axis=0).astype(np.float32)


def kernel(Q, K, V, omega, num_batch, batch_seg):
    nc = _get_nc()
    in_maps = make_in_maps(Q, K, V, omega)
    res = run_bass_kernel_spmd(nc, in_maps, core_ids=list(range(N_CORES)))
    return assemble(res.results)


# revision 34
# speedup vs baseline: 1.1631x; 1.1631x over previous
"""Trainium2 Bass kernel for segmented linear (performer-style) attention.

Problem: nn_Attention_43550968382196 (sparse_attention).
  N=32768 tokens in 64 contiguous equal segments of 512, d_qk=128, d_v=256,
  m=256 random features.  Per segment:
     phi_q = (exp(Uq - hq - rowmax(Uq)) + eps) / sqrt(m)
     phi_k = (exp(Uk - hk - segmax(Uk)) + eps) / sqrt(m)
     out   = (phi_q @ (phi_k^T V)) / (phi_q . sum(phi_k) + 1e-8)

Device math (v8; validated ~5e-3 rel err vs the jax reference in numpy):
  * All matmuls bf16 operands, fp32 PSUM accumulation (fp8 was tested and
    fails the 2e-2 gate: e4m3's 6% per-element error survives averaging).
  * Q side: Qp = exp(Uq - hq - mx) via one Act pass per chunk (bias AP);
    the +eps rides the PSUM->SBUF copy after the PE transpose (Copy with
    float bias / tensor_scalar_add), so no eps rank-1 on the Q side.
  * K side: e^{-hk} is folded into V' rows ON THE HOST, and V' carries a
    257th column equal to e^{-hk}, so phi_k-dev = exp(Uk) needs NO bias
    (one exp per psum tile) and Ksum drops out of the KV matmul as column
    256 -- no separate Ksum or den matmul chains.  The segment max enters
    only through the eps correction: segmax' = max(exp(Uk)) = e^{segmax}
    via a gpsimd all-dims reduce, used as the rank-1 lhsT scale:
    KV += segmax' * (eps * [colsum_raw(V) | 512]).  The per-segment scale
    e^{segmax} cancels in the num/den ratio.
  * num[:, 0:256] and den = num[:, 256] leave as one bf16 [512, 257] DMA
    per segment; the division (+ the 1e-8*m*segmax' norm epsilon) happens
    on the host.
  * ~5us of dummy matmuls at t=0 ramp the PE p-state to 2.4GHz while the
    input DMAs fill SBUF.

Sharding: 64 segments split 8-per-core across 8 NeuronCores (data parallel,
no collectives); each core runs this program on its 4096-token shard.
"""

import math
import os
import sys

for _p in ("/opt/trn_rl_repo",):
    if _p not in sys.path and os.path.isdir(_p):
        sys.path.insert(0, _p)

import numpy as np
import ml_dtypes

import concourse.bass as bass
import concourse.bacc as bacc
import concourse.tile as tile
from concourse import mybir
from concourse.bass_utils import run_bass_kernel_spmd

F32 = mybir.dt.float32
BF16 = mybir.dt.bfloat16
AF = mybir.ActivationFunctionType
ALU = mybir.AluOpType
AX = mybir.AxisListType

N_CORES = 8
N = 32768
D = 128          # qk dim
M = 256          # features
DV = 256         # v dim
DV1 = 257        # device V' columns: [V | 1] (all rows scaled by e^{-hk})
P = 128          # partitions / tokens per chunk
NSEG = 8         # segments per core
CH = 4           # chunks per segment
MC = 2           # m chunks (256 / 128)
TOK = NSEG * 512

EPS_PHI = 1e-4
EPS_NORM = 1e-8


def build_nc():
    nc = bacc.Bacc("TRN2", target_bir_lowering=False, debug=False)

    QTd = nc.declare_dram_parameter("QT", [D, TOK], BF16, isOutput=False)
    KTd = nc.declare_dram_parameter("KT", [D, TOK], BF16, isOutput=False)
    Vd = nc.declare_dram_parameter("VP", [TOK, DV1], BF16, isOutput=False)
    Wd = nc.declare_dram_parameter("omega", [D, M], BF16, isOutput=False)
    HQd = nc.declare_dram_parameter("HQM", [P, NSEG * CH], F32, isOutput=False)
    CVd = nc.declare_dram_parameter("CVS", [1, NSEG * DV1], BF16,
                                    isOutput=False)
    Id = nc.declare_dram_parameter("identb", [P, P], BF16, isOutput=False)
    Od = nc.declare_dram_parameter("num", [TOK, DV1], BF16, isOutput=True)
    Sd = nc.declare_dram_parameter("smax", [1, NSEG], F32, isOutput=True)

    Vv = Vd[:, :].rearrange("(s c p) d -> s p c d", s=NSEG, c=CH, p=P)
    Ov = Od[:, :].rearrange("(s c p) d -> s p c d", s=NSEG, c=CH, p=P)

    with tile.TileContext(nc) as tc:
        with (
            tc.tile_pool(name="const", bufs=1) as const,
            tc.tile_pool(name="sb", bufs=2) as sb,
            tc.tile_pool(name="sm", bufs=3) as sm,
            tc.tile_pool(name="ps", bufs=1, space="PSUM") as ps,
        ):
            # PE warm-up scratch (no input deps)
            scr1 = const.tile([P, 1], BF16, name="scr1")
            nc.vector.memset(scr1[:, :], 1.0)
            scr2 = const.tile([P, 512], BF16, name="scr2")
            nc.vector.memset(scr2[:, :], 1.0)

            # omega + segment-0 inputs first, then consts, then bulk rest
            qT_all = const.tile([D, TOK], BF16, name="qT_all")
            kT_all = const.tile([D, TOK], BF16, name="kT_all")
            vp_all = const.tile([P, NSEG, CH, DV1], BF16, name="vp_all")
            omega_t = const.tile([D, M], BF16, name="omega_t")
            nc.sync.dma_start(omega_t[:, :], Wd[:, :])
            nc.sync.dma_start(qT_all[:, 0:512], QTd[:, 0:512])
            nc.sync.dma_start(kT_all[:, 0:512], KTd[:, 0:512])
            ident_t = const.tile([P, P], BF16, name="ident_t")
            nc.sync.dma_start(ident_t[:, :], Id[:, :])
            nc.sync.dma_start(vp_all[:, 0], Vv[0])
            hqm_t = const.tile([P, NSEG, CH], F32, name="hqm_t")
            nc.sync.dma_start(
                hqm_t[:, :, :],
                HQd[:, :].rearrange("p (s c) -> p s c", s=NSEG))
            cvs_t = const.tile([1, NSEG, DV1], BF16, name="cvs_t")
            nc.sync.dma_start(
                cvs_t[:, :, :],
                CVd[:, :].rearrange("p (s d) -> p s d", s=NSEG))
            smaxAll = const.tile([1, NSEG], F32, name="smaxAll")

            # remaining per-segment loads (keeps early segments' data close)
            for s in range(1, NSEG):
                sl = bass.ts(s, 512)
                nc.sync.dma_start(kT_all[:, sl], KTd[:, sl])
                nc.sync.dma_start(qT_all[:, sl], QTd[:, sl])
                nc.sync.dma_start(vp_all[:, s], Vv[s])

            # warm-up matmuls during the DMA fill (output unread)
            warm = ps.tile([P, DV1], F32, name="warm", tag="NN", bufs=2)
            for i in range(12):
                nc.tensor.matmul(warm[0:1, :], scr1[:, 0:1],
                                 scr2[:, 0:DV1], skip_group_check=True)

            # per-segment state carried between pipeline stages
            stK = [None] * NSEG
            st = [None] * NSEG

            def stageK(s):
                # ---- K side, run 2 segments ahead: hides the smax
                # chain (expK -> gpsimd 2us -> smrow) completely -------
                uk0 = ps.tile([P, 2, M], F32, name=f"uk0_{s}", tag="U", bufs=4)
                uk1 = ps.tile([P, 2, M], F32, name=f"uk1_{s}", tag="U", bufs=4)
                for c in range(CH):
                    u = (uk0, uk1)[c // 2]
                    nc.tensor.matmul(u[:, c % 2, :],
                                     kT_all[:, bass.ts(s * CH + c, P)],
                                     omega_t[:, :])
                # K: exp with no bias (one op per psU tile)
                kp = sb.tile([P, CH, M], BF16, name=f"kp{s}", tag="kp", bufs=4)
                nc.scalar.activation(kp[:, 0:2, :], uk0[:, :, :], AF.Exp)
                nc.scalar.activation(kp[:, 2:4, :], uk1[:, :, :], AF.Exp)
                # segmax' = max(exp(Uk)) via gpsimd all-reduce (SBUF in)
                smx = sm.tile([1, 1], F32, name=f"smx{s}", tag="smx")
                nc.gpsimd.tensor_reduce(smx[:, :], kp[:, :, :],
                                        axis=AX.XYZWC, op=ALU.max)
                smrow = sm.tile([1, P], BF16, name=f"smrow{s}", tag="smrow")
                nc.vector.tensor_copy(smrow[:, :],
                                      smx[:, :].broadcast_to([1, P]))
                nc.gpsimd.tensor_copy(smaxAll[0:1, s:s + 1], smx[:, :])
                stK[s] = (kp, smrow)

            def stage1_mm(s):
                # ---- Q side: U matmuls, rowmax -> bias ---------------
                uq0 = ps.tile([P, 2, M], F32, name=f"uq0_{s}", tag="U", bufs=4)
                uq1 = ps.tile([P, 2, M], F32, name=f"uq1_{s}", tag="U", bufs=4)
                for c in range(CH):
                    u = (uq0, uq1)[c // 2]
                    nc.tensor.matmul(u[:, c % 2, :],
                                     qT_all[:, bass.ts(s * CH + c, P)],
                                     omega_t[:, :])
                mx4 = sm.tile([P, CH], F32, name=f"mx4_{s}", tag="mx4")
                nc.vector.tensor_reduce(mx4[:, 0:2], uq0[:, :, :],
                                        axis=AX.X, op=ALU.max)
                nc.vector.tensor_reduce(mx4[:, 2:4], uq1[:, :, :],
                                        axis=AX.X, op=ALU.max)
                biasq = sm.tile([P, CH], F32, name=f"biasq_{s}", tag="biasq")
                nc.gpsimd.tensor_tensor(biasq[:, :], hqm_t[:, s], mx4[:, :],
                                        op=ALU.subtract)
                st[s] = (uq0, uq1, biasq)

            def stage1_exp(s):
                uq0, uq1, biasq = st[s]
                qp = sb.tile([P, CH, M], BF16, name=f"qp{s}", tag="qp", bufs=3)
                for c in range(CH):
                    nc.scalar.activation(qp[:, c, :],
                                         (uq0, uq1)[c // 2][:, c % 2, :],
                                         AF.Exp, bias=biasq[:, c:c + 1])
                st[s] = qp

            qpTs = [None] * NSEG
            kvbs = [None] * NSEG

            def stage2a_T(s):
                qp = st[s]
                # ---- QpT = T(qp) + eps  (PE transpose, copy adds eps) -
                psT = ps.tile([P, MC, 512], BF16, name=f"psT_{s}", tag="T",
                              bufs=1)
                for c in range(CH):
                    nc.tensor.transpose(psT[:, 0, bass.ts(c, P)],
                                        qp[:, c, 0:P], ident_t[:, :])
                    nc.tensor.transpose(psT[:, 1, bass.ts(c, P)],
                                        qp[:, c, P:M], ident_t[:, :])
                qpT = sb.tile([P, MC, 512], BF16, name=f"qpT{s}", tag="qpT",
                              bufs=2)
                nc.scalar.activation(qpT[:, 0, :], psT[:, 0, :], AF.Copy,
                                     bias=EPS_PHI)
                nc.vector.tensor_scalar_add(qpT[:, 1, :], psT[:, 1, :],
                                            EPS_PHI)
                qpTs[s] = qpT
                kvbs[s] = sb.tile([P, MC, DV1], BF16, name=f"kvb{s}",
                                  tag="kvb", bufs=2)

            def stage2a_KV(s, mc):
                kp, smrow = stK[s]
                # ---- KV = Kp^T [V'|e^{-hk}] (+ rank-1 eps, first) -----
                psKV = ps.tile([P, DV1], F32, name=f"psKV{s}_{mc}",
                               tag="W", bufs=1)
                nc.tensor.matmul(psKV[:, :], smrow[0:1, :],
                                 cvs_t[0:1, s, :], start=True, stop=False)
                for c in range(CH):
                    nc.tensor.matmul(psKV[:, :],
                                     kp[:, c, bass.ts(mc, P)],
                                     vp_all[:, s, c, :],
                                     start=False, stop=(c == CH - 1))
                if mc == 0:
                    nc.vector.tensor_copy(kvbs[s][:, 0, :], psKV[:, :])
                else:
                    nc.scalar.activation(kvbs[s][:, 1, :], psKV[:, :],
                                         AF.Copy)

            def stage2b(s):
                qpT, kvb = qpTs[s], kvbs[s]
                # ---- num matmuls ([t, V'|den] per chunk) + store ------
                for half in range(2):
                    numb = sb.tile([P, 2, DV1], BF16,
                                   name=f"numb{s}_{half}", tag="numb",
                                   bufs=3)
                    for i in range(2):
                        c = half * 2 + i
                        psN = ps.tile([P, DV1], F32, name=f"psN{s}_{c}",
                                      tag="NN", bufs=2)
                        for mc in range(MC):
                            nc.tensor.matmul(psN[:, :],
                                             qpT[:, mc, bass.ts(c, P)],
                                             kvb[:, mc, :],
                                             start=(mc == 0), stop=(mc == 1))
                        if s == NSEG - 1 and i == 0:
                            # tail: parallelize the last segment's copies
                            nc.scalar.activation(numb[:, i, :], psN[:, :],
                                                 AF.Copy)
                        else:
                            nc.vector.tensor_copy(numb[:, i, :], psN[:, :])
                    nc.sync.dma_start(Ov[s, :, 2 * half:2 * half + 2, :],
                                      numb[:, :, :])

            # 2.5-deep software pipeline: iteration s emits Q-matmuls for
            # s+1, K side for s+2, transposes+KV for s, num for s-1 -- so
            # every PE instruction's deps are >= 1 iteration old.
            # Prefix: Q side first so biasq(0) isn't queued behind the
            # 2us gpsimd segmax reduce on Pool, and expQ(0) leads Act.
            stage1_mm(0)
            stage1_exp(0)
            stageK(0)
            stageK(1)
            for s in range(NSEG):
                if s + 1 < NSEG:
                    stage1_mm(s + 1)
                if s + 2 < NSEG:
                    stageK(s + 2)
                stage2a_T(s)
                # KV mc=1 early: its Act-side kvb copy gates the W-ring
                # recycle for the next segment, so don't queue it behind
                # the expQ block.
                stage2a_KV(s, 1)
                if s + 1 < NSEG:
                    stage1_exp(s + 1)
                stage2a_KV(s, 0)
                if s > 0:
                    stage2b(s - 1)
            stage2b(NSEG - 1)

            nc.sync.dma_start(Sd[:, :], smaxAll[:, :])

    nc.compile()
    return nc


_NC_CACHE = {}


def _get_nc():
    if "nc" not in _NC_CACHE:
        _NC_CACHE["nc"] = build_nc()
    return _NC_CACHE["nc"]


def make_in_maps(Q, K, V, omega):
    bf = ml_dtypes.bfloat16
    Q = np.ascontiguousarray(np.asarray(Q, dtype=np.float32))
    K = np.ascontiguousarray(np.asarray(K, dtype=np.float32))
    V = np.ascontiguousarray(np.asarray(V, dtype=np.float32))
    omega = np.asarray(omega, dtype=np.float32)

    QT = Q.T.astype(bf)
    KT = K.T.astype(bf)
    omega_s = (omega * np.float32(D ** -0.25)).astype(bf)
    hscale = np.float32(1.0 / (2.0 * math.sqrt(D)))
    hq = (Q * Q).sum(axis=1) * hscale            # [N]
    hk = (K * K).sum(axis=1) * hscale
    ehk = np.exp(-hk).astype(np.float32)          # [N]
    Vb = V.astype(bf).astype(np.float32)
    # V' = e^{-hk} * [V | 1]  (column 256 = e^{-hk} itself)
    VP = (ehk[:, None] * np.concatenate(
        [Vb, np.ones((N, 1), np.float32)], axis=1)).astype(bf)
    # eps * per-segment colsum of raw [V | 1] (bf16-rounded V)
    nseg_tot = N_CORES * NSEG
    cvs = np.concatenate(
        [EPS_PHI * Vb.reshape(nseg_tot, 512, DV).sum(axis=1),
         np.full((nseg_tot, 1), EPS_PHI * 512.0, np.float32)],
        axis=1).astype(bf)                        # [nseg, 257]
    ident = np.eye(P, dtype=np.float32).astype(bf)

    hqm = np.ascontiguousarray(
        (-hq).reshape(N_CORES, NSEG, CH, P).transpose(0, 3, 1, 2)
        .reshape(N_CORES, P, NSEG * CH)).astype(np.float32)

    in_maps = []
    for c in range(N_CORES):
        sl = slice(c * TOK, (c + 1) * TOK)
        in_maps.append({
            "QT": np.ascontiguousarray(QT[:, sl]),
            "KT": np.ascontiguousarray(KT[:, sl]),
            "VP": VP[sl],
            "omega": omega_s,
            "HQM": hqm[c],
            "CVS": np.ascontiguousarray(
                cvs[c * NSEG:(c + 1) * NSEG].reshape(1, NSEG * DV1)),
            "identb": ident,
        })
    return in_maps


def assemble(results):
    outs = []
    for c in range(N_CORES):
        r = results[c]
        num = np.asarray(r["num"], dtype=np.float32)          # [TOK, 257]
        smax = np.asarray(r["smax"], dtype=np.float32).reshape(NSEG)
        den = num[:, DV] + (M * EPS_NORM) * np.repeat(smax, 512)
        outs.append(num[:, 0:DV] / den[:, None])
    return np.concatenate(outs, axis=0).astype(np.float32)


def kernel(Q, K, V, omega, num_batch, batch_seg):
    nc = _get_nc()
    in_maps = make_in_maps(Q, K, V, omega)
    res = run_bass_kernel_spmd(nc, in_maps, core_ids=list(range(N_CORES)))
    return assemble(res.results)
